# revision 29
# baseline (speedup 1.0000x reference)
"""MemoryCompressedAttention Trainium2 kernel (8-core SPMD).

Sharding: core c handles batch b = c // 2 and head-group hg = c % 2
(8 of 16 heads = a 512-wide slice of the d_model head space). Each core
computes its batch/head-group block end-to-end (conv+K/V-proj fused into
one 3072-deep GEMM on the host-packed weights); the host sums the two
head-group partial outputs per batch.

Performance notes (595us -> ~548us on hw):
  - Host pre-packs every DMA'd tensor so each transfer is CONTIGUOUS
    per partition (kr/vr/q/wck in chunk-major [chunk][p][...] blocks).
    Strided "(ci p) l -> p ci l" rearranges made the Sync engine spend
    1-3.5us programming descriptors per transfer, serializing startup.
  - Q-projection runs in fp8e4 DoubleRow perf mode (256-deep
    contraction per pass: 8 -> 4 matmuls per head-tile, q/wq DMA
    halved). fp8 on the q path costs ~5e-3 rel err (total 8.9e-3 vs
    the 2e-2 gate); k/v/o paths must stay bf16 -- their fp8 error
    lands directly on the output. Scores/AV can't use DoubleRow: PE
    time is out-cols-bound and DR only pays for deep contractions.
  - V stationaries are [kl, 128] with cols 0:64 all-ones: the softmax
    denominator lands PRE-BROADCAST on PSUM partitions 0:63, so the
    out-path is just recip(PSUM)+mult on DVE (no staging copy, no
    GpSimd partition_broadcast, no single-partition ops).
  - outT is bf16 (halves the output write; host sums in fp32).
  - Fillers (Q-proj/O-proj/wo-DMA) pop INSIDE the attention units'
    klt loops (pop_mod=2) so the per-klt ACT slack absorbs projection
    work; the first three q-chunks are projected inline at the head
    while kr/wck stream in; kr chunks 0-2 are prefetched; the last
    q-chunk's O-proj chains alternate PSUM pools for a 4-deep
    drain pipeline.
"""

import numpy as np
import ml_dtypes

B, S, D, H, DK, CR = 4, 4096, 1024, 16, 64, 3
PAD = CR - D % CR          # 2
KL = (S + PAD) // CR       # 1366 compressed rows
CD = CR * D                # 3072 fused contraction dim
HGD = 512                  # per-core head-group width (8 heads x 64)
NKC = CD // 128            # 24 contraction chunks of 128
N_KLT = (KL + 127) // 128  # 11 kl row-tiles (last one is 86 rows)
KLP = N_KLT * 128          # 1408: kl padded to full tiles
NQC = S // 512             # 8 q column chunks
KCH = 256                  # kr chunk cols
NKCH = (KL + KCH - 1) // KCH

bf16 = ml_dtypes.bfloat16
e4m3 = ml_dtypes.float8_e4m3fn

_CACHE = {}


def _build_nc(lag=3, pt_bufs=5, pss_bufs=2, proj_bufs=2, q_ahead=6,
              kr_bufs=3, vr_bufs=2, stg_bufs=1, pop_mod=2, pop_pre=False,
              pop_n=2, q_dr=True):
    import concourse.bass as bass
    import concourse.tile as tile
    from concourse import bacc
    from concourse import mybir
    from contextlib import ExitStack

    f32 = mybir.dt.float32
    bf = mybir.dt.bfloat16
    f8 = mybir.dt.float8e4
    EXP = mybir.ActivationFunctionType.Exp
    DR = mybir.MatmulPerfMode.DoubleRow

    nc = bacc.Bacc(None)

    # chunk-major host-packed layouts: one contiguous row-block per use
    qTn = nc.declare_dram_parameter("qTn", [NQC * 128, 8 * 512], f8,
                                    isOutput=False)
    krTn = nc.declare_dram_parameter("krTn", [NKCH * 128, NKC * KCH], bf,
                                     isOutput=False)
    vrTn = nc.declare_dram_parameter("vrTn", [N_KLT * 128, NKC * 128], bf,
                                     isOutput=False)
    wckTn = nc.declare_dram_parameter("wckTn", [4 * 128, NKC * 128], bf,
                                      isOutput=False)
    wcvTn = nc.declare_dram_parameter("wcvTn", [128, NKC * HGD], bf,
                                      isOutput=False)
    wqTn = nc.declare_dram_parameter("wqTn", [128, 8 * HGD], f8,
                                     isOutput=False)
    woTn = nc.declare_dram_parameter("woTn", [128, 4 * D], bf,
                                     isOutput=False)
    bqf = nc.declare_dram_parameter("bqf", [HGD, 1], f32, isOutput=False)
    bkf = nc.declare_dram_parameter("bkf", [HGD, 1], f32, isOutput=False)
    bvf = nc.declare_dram_parameter("bvf", [1, HGD], f32, isOutput=False)
    outT = nc.declare_dram_parameter("outT", [D, S], bf, isOutput=True)

    with tile.TileContext(nc) as tc, ExitStack() as ctx:
        persist = ctx.enter_context(tc.tile_pool(name="persist", bufs=1))
        krp = ctx.enter_context(tc.tile_pool(name="krp", bufs=kr_bufs))
        vrp = ctx.enter_context(tc.tile_pool(name="vrp", bufs=vr_bufs))
        qstream = ctx.enter_context(tc.tile_pool(name="qstream", bufs=4))
        qtp = ctx.enter_context(tc.tile_pool(name="qtp", bufs=q_ahead + 1))
        osbp = ctx.enter_context(tc.tile_pool(name="osbp", bufs=2))
        ptp = ctx.enter_context(tc.tile_pool(name="ptp", bufs=pt_bufs))
        stgp = ctx.enter_context(tc.tile_pool(name="stgp", bufs=stg_bufs))
        otp = ctx.enter_context(tc.tile_pool(name="otp", bufs=3))
        psA = ctx.enter_context(
            tc.tile_pool(name="psA", bufs=pss_bufs, space="PSUM"))
        psP = ctx.enter_context(
            tc.tile_pool(name="psP", bufs=proj_bufs, space="PSUM"))
        psO = ctx.enter_context(
            tc.tile_pool(name="psO", bufs=1, space="PSUM"))

        # ---- persistent tiles ----
        ktT_sb = persist.tile([128, 4, KLP], bf)
        # vones: [kl(128), klt, sub-head(8), 128]: cols 0:64 = 1, 64:128 = V
        vones_sb = persist.tile([128, N_KLT, 8, 128], bf)
        wck_sb = persist.tile([128, 4, NKC, 128], bf)   # ht-major
        wcv_sb = persist.tile([128, NKC, HGD], bf)
        wq_sb = persist.tile([128, 8, HGD], f8)
        wo_sb = persist.tile([128, 4, D], bf)
        bk_sb = persist.tile([128, 4, 1], f32)
        bq_sb = persist.tile([128, 4, 1], f32)
        bvb_sb = persist.tile([128, HGD], f32)

        qt_tiles = {}

        def emit_q_dma(qc):
            q_sb = qstream.tile([128, 8, 512], f8, tag="q", name="q_sb")
            nc.sync.dma_start(
                out=q_sb,
                in_=qTn[qc * 128:(qc + 1) * 128, :].rearrange(
                    "p (i s) -> p i s", i=8))
            qt = qtp.tile([128, 4, 512], bf, tag="qt", name="qt")
            qt_tiles[qc] = (qt, q_sb)

        def emit_q_ht(qc, ht):
            qt, q_sb = qt_tiles[qc]
            psq = psP.tile([128, 512], f32, tag="proj", name="psq")
            if q_dr:
                for j in range(4):
                    nc.tensor.matmul(
                        psq,
                        wq_sb[:, 2 * j:2 * j + 2, ht * 128:(ht + 1) * 128],
                        q_sb[:, 2 * j:2 * j + 2, :],
                        start=(j == 0), stop=(j == 3), perf_mode=DR)
            else:
                for dm in range(8):
                    nc.tensor.matmul(
                        psq, wq_sb[:, dm, ht * 128:(ht + 1) * 128],
                        q_sb[:, dm, :], start=(dm == 0), stop=(dm == 7))
            nc.vector.tensor_scalar_add(
                qt[:, ht, :], psq, bq_sb[:, ht, :])

        def emit_kr_dma(ch):
            krb = krp.tile([128, NKC, KCH], bf, tag="kr", name="krb")
            nc.sync.dma_start(
                out=krb,
                in_=krTn[ch * 128:(ch + 1) * 128, :].rearrange(
                    "p (ci l) -> p ci l", ci=NKC))
            return krb

        def emit_wck_dma(ht):
            nc.sync.dma_start(
                out=wck_sb[:, ht],
                in_=wckTn[ht * 128:(ht + 1) * 128, :].rearrange(
                    "p (ci o) -> p ci o", ci=NKC))

        # ---- startup DMA order: cheap contiguous issues, critical first
        nc.sync.dma_start(
            out=wq_sb, in_=wqTn[:, :].rearrange("p (i s) -> p i s", i=8))
        emit_q_dma(0)
        nc.sync.dma_start(
            out=bq_sb, in_=bqf.rearrange("(t p) o -> p t o", p=128))
        kr_pf = {0: emit_kr_dma(0)}
        emit_wck_dma(0)
        emit_q_dma(1)
        kr_pf[1] = emit_kr_dma(1)
        for ht in range(1, 4):
            emit_wck_dma(ht)
        emit_q_dma(2)
        emit_q_dma(3)
        nc.sync.dma_start(
            out=bk_sb, in_=bkf.rearrange("(t p) o -> p t o", p=128))
        nc.sync.dma_start(out=bvb_sb, in_=bvf[0:1, :].partition_broadcast(128))
        kr_pf[2] = emit_kr_dma(2)

        for ht in range(4):
            emit_q_ht(0, ht)
        for ht in range(4):
            emit_q_ht(1, ht)
        for ht in range(4):
            emit_q_ht(2, ht)
        for ht in range(4):
            emit_q_ht(3, ht)

        nc.vector.memset(vones_sb[:, :, :, 0:64], 1.0)
        nc.vector.memset(ktT_sb[:, :, KL:KLP], 0.0)  # kl padding

        def emit_wcv_dma():
            # quarters so V(klt0)'s ci-chain can start early
            for c0 in range(0, NKC, 6):
                nc.sync.dma_start(
                    out=wcv_sb[:, c0:c0 + 6, :],
                    in_=wcvTn[:, c0 * HGD:(c0 + 6) * HGD].rearrange(
                        "p (ci o) -> p ci o", ci=6))

        def emit_wo_dma():
            nc.sync.dma_start(
                out=wo_sb, in_=woTn[:, :].rearrange("p (c d) -> p c d", c=4))

        # ---- K production: one KCH-col chunk, selectable head tiles ----
        kr_live = {}

        def emit_k(ch, hts=(0, 1, 2, 3)):
            kc0 = ch * KCH
            kcn = min(KCH, KL - kc0)
            if ch in kr_live:
                krb = kr_live[ch]
            elif ch in kr_pf:
                krb = kr_live[ch] = kr_pf.pop(ch)
            else:
                krb = kr_live[ch] = emit_kr_dma(ch)
            for ht in hts:
                psk = psP.tile([128, 512], f32, tag="proj", name="psk")
                for ci in range(NKC):
                    nc.tensor.matmul(
                        psk[:, :kcn],
                        wck_sb[:, ht, ci, :],
                        krb[:, ci, :kcn],
                        start=(ci == 0), stop=(ci == NKC - 1))
                nc.vector.tensor_scalar_add(
                    ktT_sb[:, ht, kc0:kc0 + kcn],
                    psk[:, :kcn], bk_sb[:, ht, :])

        def emit_v(klt):
            rn = min(128, KL - klt * 128)
            vrb = vrp.tile([128, NKC, 128], bf, tag="vr", name="vrb")
            nc.sync.dma_start(
                out=vrb,
                in_=vrTn[klt * 128:(klt + 1) * 128, :].rearrange(
                    "p (ci l) -> p ci l", ci=NKC))
            psv = psP.tile([128, 512], f32, tag="proj", name="psv")
            for ci in range(NKC):
                nc.tensor.matmul(
                    psv[:rn, :], vrb[:, ci, :rn], wcv_sb[:, ci, :],
                    start=(ci == 0), stop=(ci == NKC - 1))
            nc.vector.tensor_tensor(
                out=vones_sb[:rn, klt, :, 64:128],
                in0=psv[:rn].rearrange("p (h c) -> p h c", h=8),
                in1=bvb_sb[:rn].rearrange("p (h c) -> p h c", h=8),
                op=mybir.AluOpType.add)

        # K/V emission cursors for just-in-time production
        kv_state = {"k": 0, "v": 0, "k0_rest": False}

        def need_k(ch):
            if kv_state["k"] == 0:
                emit_k(0, hts=(0,))
                kv_state["k"] = 1
            if ch >= 1 and not kv_state["k0_rest"]:
                emit_k(0, hts=(1, 2, 3))
                del kr_live[0]
                kv_state["k0_rest"] = True
            while kv_state["k"] <= min(ch, NKCH - 1):
                emit_k(kv_state["k"])
                del kr_live[kv_state["k"]]
                kv_state["k"] += 1

        def need_v(klt):
            if kv_state["v"] == 0:
                emit_wcv_dma()
            while kv_state["v"] <= min(klt, N_KLT - 1):
                emit_v(kv_state["v"])
                kv_state["v"] += 1

        # ---- attention unit: head pair hp x q-chunk qc ----
        def emit_unit(hp, qc, qt, osb_t, kv_jit=False, pop=None):
            ht = hp
            psos = [psO.tile([128, 512], f32, tag=f"pso{s}", name=f"pso{s}")
                    for s in range(2)]
            pts = {}

            def emit_av(k):
                rn = min(128, KL - k * 128)
                pt = pts.pop(k)
                for sub in range(2):
                    nc.tensor.matmul(
                        psos[sub],
                        vones_sb[:rn, k, 2 * hp + sub, :],
                        pt[:rn, sub * 512:(sub + 1) * 512],
                        start=(k == 0), stop=(k == N_KLT - 1))

            for klt in range(N_KLT):
                if kv_jit:
                    need_k(klt * 128 // KCH)
                if pop_pre and pop is not None and klt % pop_mod == 1:
                    pop()
                pss = psA.tile([128, 1024], f32, tag="pss", name="pss")
                for sub in range(2):
                    hb = sub * 64
                    nc.tensor.matmul(
                        pss[:, sub * 512:(sub + 1) * 512],
                        ktT_sb[hb:hb + 64, ht,
                               klt * 128:(klt + 1) * 128],
                        qt[hb:hb + 64, ht, :],
                        start=True, stop=True)
                pt = ptp.tile([128, 1024], bf, tag="pt", name="pt")
                nc.scalar.activation(pt, pss, EXP, scale=0.125)
                pts[klt] = pt
                if klt >= lag:
                    if kv_jit:
                        need_v(klt - lag)
                    emit_av(klt - lag)
                if not pop_pre and pop is not None and klt % pop_mod == 1:
                    pop()
            if kv_jit:
                need_v(N_KLT - 1)
            for k in range(max(0, N_KLT - lag), N_KLT):
                emit_av(k)

            for sub in range(2):
                hb = sub * 64
                # den is pre-broadcast on psum partitions 0:64 (ones cols
                # lead); x sits on partitions 64:128. recip reads PSUM
                # base-0 directly, so the staging copy is gone.
                rcb = stgp.tile([64, 512], f32, tag=f"rcb{sub}",
                                name=f"rcb{sub}")
                nc.vector.reciprocal_approx_fast(out=rcb,
                                                 in_=psos[sub][0:64, :])
                nc.vector.tensor_tensor(
                    out=osb_t[hb:hb + 64, ht, :],
                    in0=psos[sub][64:128, :], in1=rcb,
                    op=mybir.AluOpType.mult)

        # ---- fine-grained filler closures ----
        def o_filler_closures(qc, osb_t, alt_pools=False):
            qsl = slice(qc * 512, (qc + 1) * 512)
            state = {}

            def mk_mm(dt, hc):
                def f():
                    if hc == 0:
                        if alt_pools and dt % 2 == 1:
                            state[dt] = psO.tile(
                                [128, 512], f32,
                                tag=f"pso{(dt // 2) % 2}", name="pp8")
                        else:
                            state[dt] = psP.tile([128, 512], f32,
                                                 tag="proj", name="pp")
                    nc.tensor.matmul(
                        state[dt], wo_sb[:, hc, dt * 128:(dt + 1) * 128],
                        osb_t[:, hc, :], start=(hc == 0), stop=(hc == 3))
                return f

            def mk_drain(dt):
                def f():
                    ot = otp.tile([128, 512], bf, tag="ot", name="ot")
                    nc.vector.tensor_copy(ot, state.pop(dt))
                    nc.sync.dma_start(
                        out=outT[dt * 128:(dt + 1) * 128, qsl], in_=ot)
                return f

            out = []
            for dt in range(8):
                for hc in range(4):
                    out.append(mk_mm(dt, hc))
                out.append(mk_drain(dt))
            return out

        def q_filler_closures(qc):
            state = {}
            NJ = 4 if q_dr else 8

            def mk_mm(ht, j):
                def f():
                    if j == 0:
                        state[ht] = psP.tile([128, 512], f32, tag="proj",
                                             name="psq")
                    if q_dr:
                        nc.tensor.matmul(
                            state[ht],
                            wq_sb[:, 2 * j:2 * j + 2,
                                  ht * 128:(ht + 1) * 128],
                            qt_tiles[qc][1][:, 2 * j:2 * j + 2, :],
                            start=(j == 0), stop=(j == NJ - 1),
                            perf_mode=DR)
                    else:
                        nc.tensor.matmul(
                            state[ht], wq_sb[:, j, ht * 128:(ht + 1) * 128],
                            qt_tiles[qc][1][:, j, :],
                            start=(j == 0), stop=(j == NJ - 1))
                return f

            def mk_drain(ht):
                def f():
                    nc.vector.tensor_scalar_add(
                        qt_tiles[qc][0][:, ht, :], state.pop(ht),
                        bq_sb[:, ht, :])
                    if ht == 3:
                        q_done[qc] = True
                return f

            out = [] if qc in qt_tiles else [lambda: emit_q_dma(qc)]
            for ht in range(4):
                for j in range(NJ):
                    out.append(mk_mm(ht, j))
                out.append(mk_drain(ht))
            return out

        # ---- main stream: units with O/Q filler weaving ----
        from collections import deque
        fillers = deque()
        q_done = {0: True, 1: True, 2: True, 3: True}

        def pop():
            for _ in range(pop_n):
                if fillers:
                    fillers.popleft()()

        need_k(0)
        fillers.append(emit_wo_dma)
        fillers.extend(q_filler_closures(4))
        fillers.extend(q_filler_closures(5))
        fillers.extend(q_filler_closures(6))

        for qc in range(NQC):
            while not q_done.get(qc):
                pop()
            osb_t = osbp.tile([128, 4, 512], bf, tag="osb", name="osb")
            for hp in range(4):
                kv_jit = (qc == 0 and hp == 0)
                emit_unit(hp, qc, qt_tiles[qc][0], osb_t,
                          kv_jit=kv_jit, pop=pop)
            for f in o_filler_closures(qc, osb_t,
                                       alt_pools=(qc == NQC - 1)):
                fillers.append(f)
            nq = qc + q_ahead + 1
            if nq < NQC:
                fillers.extend(q_filler_closures(nq))
        while fillers:
            pop()

    nc.finalize()
    return nc


def _host_inputs(inputs):
    """Build the 8 per-core input maps from full fp32 inputs."""
    q32 = np.asarray(inputs["query"], np.float32)
    k32 = np.asarray(inputs["key"], np.float32)
    v32 = np.asarray(inputs["value"], np.float32)
    Wq, bq = np.asarray(inputs["Wq"], np.float32), np.asarray(inputs["bq"], np.float32)
    Wk, bk = np.asarray(inputs["Wk"], np.float32), np.asarray(inputs["bk"], np.float32)
    Wv, bv = np.asarray(inputs["Wv"], np.float32), np.asarray(inputs["bv"], np.float32)
    Wo = np.asarray(inputs["Wo"], np.float32)
    conv_w = np.asarray(inputs["conv_w"], np.float32)
    conv_b = np.asarray(inputs["conv_b"], np.float32)

    Wc = conv_w.transpose(2, 1, 0).reshape(CD, D)  # [3072, 1024]

    per_hg = []
    for hg in range(2):
        hsl = slice(hg * HGD, (hg + 1) * HGD)
        wck = (Wc @ Wk[hsl].T).astype(bf16)    # [CD, HGD]
        wcv = (Wc @ Wv[hsl].T).astype(bf16)
        wq = Wq[hsl].T.astype(e4m3)            # [D, HGD]
        wo = Wo[:, hsl].T.astype(bf16)         # [HGD, D]
        # wckTn [4*128, NKC*128]: row ht*128+p, col ci*128+o
        wckTn = np.ascontiguousarray(
            wck.reshape(NKC, 128, 4, 128).transpose(2, 1, 0, 3)
            .reshape(4 * 128, NKC * 128))
        # wcvTn [128, NKC*HGD]: row p, col ci*HGD+o
        wcvTn = np.ascontiguousarray(
            wcv.reshape(NKC, 128, HGD).transpose(1, 0, 2)
            .reshape(128, NKC * HGD))
        # wqTn [128, 8*HGD]: row p, col dm*HGD+o
        wqTn = np.ascontiguousarray(
            wq.reshape(8, 128, HGD).transpose(1, 0, 2).reshape(128, 8 * HGD))
        # woTn [128, 4*D]: row p, col c*D+d
        woTn = np.ascontiguousarray(
            wo.reshape(4, 128, D).transpose(1, 0, 2).reshape(128, 4 * D))
        per_hg.append(dict(
            wckTn=wckTn, wcvTn=wcvTn, wqTn=wqTn, woTn=woTn,
            bqf=bq[hsl].reshape(HGD, 1).astype(np.float32),
            bkf=(conv_b @ Wk[hsl].T + bk[hsl]).reshape(HGD, 1).astype(np.float32),
            bvf=(conv_b @ Wv[hsl].T + bv[hsl]).reshape(1, HGD).astype(np.float32),
        ))

    per_b = []
    zpad = np.zeros((PAD, D), np.float32)
    for b in range(B):
        xr_k = np.concatenate([zpad, k32[b]], 0).reshape(KL, CD)
        xr_v = np.concatenate([zpad, v32[b]], 0).reshape(KL, CD)
        # krTn [NKCH*128, NKC*KCH]: row ch*128+p, col ci*KCH+l
        # (kr.T is [CD, KL] with row ci*128+p)
        krT = xr_k.T.astype(bf16)              # [CD, KL]
        krpk = np.zeros((NKCH * 128, NKC * KCH), bf16)
        for ch in range(NKCH):
            kcn = min(KCH, KL - ch * KCH)
            blk = krT[:, ch * KCH:ch * KCH + kcn].reshape(NKC, 128, kcn)
            krpk[ch * 128:(ch + 1) * 128]\
                .reshape(128, NKC, KCH)[:, :, :kcn] = blk.transpose(1, 0, 2)
        vrT = xr_v.T.astype(bf16)
        vrpk = np.zeros((N_KLT * 128, NKC * 128), bf16)
        for klt in range(N_KLT):
            rn = min(128, KL - klt * 128)
            blk = vrT[:, klt * 128:klt * 128 + rn].reshape(NKC, 128, rn)
            vrpk[klt * 128:(klt + 1) * 128]\
                .reshape(128, NKC, 128)[:, :, :rn] = blk.transpose(1, 0, 2)
        # qTn [NQC*128, 8*512]: row qc*128+p, col dm*512+s
        qT = q32[b].T.astype(e4m3)             # [D, S]
        qTn = np.ascontiguousarray(
            qT.reshape(8, 128, NQC, 512).transpose(2, 1, 0, 3)
            .reshape(NQC * 128, 8 * 512))
        per_b.append(dict(qTn=qTn, krTn=krpk, vrTn=vrpk))

    in_maps = []
    for c in range(8):
        b, hg = c // 2, c % 2
        in_maps.append({**per_b[b], **per_hg[hg]})
    return in_maps


def kernel(**inputs):
    from concourse.bass_utils import run_bass_kernel_spmd

    if "nc" not in _CACHE:
        _CACHE["nc"] = _build_nc()
    nc = _CACHE["nc"]

    in_maps = _host_inputs(inputs)
    r = run_bass_kernel_spmd(nc, in_maps, list(range(8)))
    _CACHE["exec_time_ns"] = r.exec_time_ns
    _CACHE["result"] = r
    res = r.results

    bo = np.asarray(inputs["bo"], np.float32)
    out = np.empty((B, S, D), np.float32)
    for b in range(B):
        out[b] = (res[2 * b]["outT"].astype(np.float32).T
                  + res[2 * b + 1]["outT"].astype(np.float32).T + bo)
    return out


# revision 32
# speedup vs baseline: 1.0250x; 1.0250x over previous
"""MemoryCompressedAttention Trainium2 kernel (8-core SPMD).

Sharding: core c handles batch b = c // 2 and head-group hg = c % 2
(8 of 16 heads = a 512-wide slice of the d_model head space). Each core
computes its batch/head-group block end-to-end (conv+K/V-proj fused into
one 3072-deep GEMM on the host-packed weights); the host sums the two
head-group partial outputs per batch.

Performance notes (595us -> ~548us on hw):
  - Host pre-packs every DMA'd tensor so each transfer is CONTIGUOUS
    per partition (kr/vr/q/wck in chunk-major [chunk][p][...] blocks).
    Strided "(ci p) l -> p ci l" rearranges made the Sync engine spend
    1-3.5us programming descriptors per transfer, serializing startup.
  - Q-projection runs in fp8e4 DoubleRow perf mode (256-deep
    contraction per pass: 8 -> 4 matmuls per head-tile, q/wq DMA
    halved). fp8 on the q path costs ~5e-3 rel err (total 8.9e-3 vs
    the 2e-2 gate); k/v/o paths must stay bf16 -- their fp8 error
    lands directly on the output. Scores/AV can't use DoubleRow: PE
    time is out-cols-bound and DR only pays for deep contractions.
  - V stationaries are [kl, 128] with cols 0:64 all-ones: the softmax
    denominator lands PRE-BROADCAST on PSUM partitions 0:63, so the
    out-path is just recip(PSUM)+mult on DVE (no staging copy, no
    GpSimd partition_broadcast, no single-partition ops).
  - outT is bf16 (halves the output write; host sums in fp32).
  - Fillers (Q-proj/O-proj/wo-DMA) pop INSIDE the attention units'
    klt loops (pop_mod=2) so the per-klt ACT slack absorbs projection
    work; the first three q-chunks are projected inline at the head
    while kr/wck stream in; kr chunks 0-2 are prefetched; the last
    q-chunk's O-proj chains alternate PSUM pools for a 4-deep
    drain pipeline.
"""

import numpy as np
import ml_dtypes

B, S, D, H, DK, CR = 4, 4096, 1024, 16, 64, 3
PAD = CR - D % CR          # 2
KL = (S + PAD) // CR       # 1366 compressed rows
CD = CR * D                # 3072 fused contraction dim
HGD = 512                  # per-core head-group width (8 heads x 64)
NKC = CD // 128            # 24 contraction chunks of 128
N_KLT = (KL + 127) // 128  # 11 kl row-tiles (last one is 86 rows)
KLP = N_KLT * 128          # 1408: kl padded to full tiles
NQC = S // 512             # 8 q column chunks
KCH = 256                  # kr chunk cols
NKCH = (KL + KCH - 1) // KCH

bf16 = ml_dtypes.bfloat16
e4m3 = ml_dtypes.float8_e4m3fn

_CACHE = {}


def _build_nc(lag=3, pt_bufs=5, pss_bufs=2, proj_bufs=2, q_ahead=6,
              kr_bufs=3, vr_bufs=2, stg_bufs=1, pop_mod=2, pop_pre=False,
              pop_n=2, q_dr=True):
    import concourse.bass as bass
    import concourse.tile as tile
    from concourse import bacc
    from concourse import mybir
    from contextlib import ExitStack

    f32 = mybir.dt.float32
    bf = mybir.dt.bfloat16
    f8 = mybir.dt.float8e4
    EXP = mybir.ActivationFunctionType.Exp
    DR = mybir.MatmulPerfMode.DoubleRow

    nc = bacc.Bacc(None)

    # chunk-major host-packed layouts: one contiguous row-block per use
    qTn = nc.declare_dram_parameter("qTn", [NQC * 128, 8 * 512], f8,
                                    isOutput=False)
    krTn = nc.declare_dram_parameter("krTn", [NKCH * 128, NKC * KCH], bf,
                                     isOutput=False)
    vrTn = nc.declare_dram_parameter("vrTn", [N_KLT * 128, NKC * 128], bf,
                                     isOutput=False)
    wckTn = nc.declare_dram_parameter("wckTn", [4 * 128, NKC * 128], bf,
                                      isOutput=False)
    wcvTn = nc.declare_dram_parameter("wcvTn", [128, NKC * HGD], bf,
                                      isOutput=False)
    wqTn = nc.declare_dram_parameter("wqTn", [128, 8 * HGD], f8,
                                     isOutput=False)
    woTn = nc.declare_dram_parameter("woTn", [128, 4 * D], bf,
                                     isOutput=False)
    bqf = nc.declare_dram_parameter("bqf", [HGD, 1], f32, isOutput=False)
    bkf = nc.declare_dram_parameter("bkf", [HGD, 1], f32, isOutput=False)
    bvf = nc.declare_dram_parameter("bvf", [1, HGD], f32, isOutput=False)
    outT = nc.declare_dram_parameter("outT", [D, S], bf, isOutput=True)

    with tile.TileContext(nc) as tc, ExitStack() as ctx:
        persist = ctx.enter_context(tc.tile_pool(name="persist", bufs=1))
        krp = ctx.enter_context(tc.tile_pool(name="krp", bufs=kr_bufs))
        vrp = ctx.enter_context(tc.tile_pool(name="vrp", bufs=vr_bufs))
        qstream = ctx.enter_context(tc.tile_pool(name="qstream", bufs=4))
        qtp = ctx.enter_context(tc.tile_pool(name="qtp", bufs=q_ahead + 1))
        osbp = ctx.enter_context(tc.tile_pool(name="osbp", bufs=2))
        ptp = ctx.enter_context(tc.tile_pool(name="ptp", bufs=pt_bufs))
        stgp = ctx.enter_context(tc.tile_pool(name="stgp", bufs=stg_bufs))
        otp = ctx.enter_context(tc.tile_pool(name="otp", bufs=3))
        psA = ctx.enter_context(
            tc.tile_pool(name="psA", bufs=pss_bufs, space="PSUM"))
        psP = ctx.enter_context(
            tc.tile_pool(name="psP", bufs=proj_bufs, space="PSUM"))
        psO = ctx.enter_context(
            tc.tile_pool(name="psO", bufs=1, space="PSUM"))

        # ---- persistent tiles ----
        ktT_sb = persist.tile([128, 4, KLP], bf)
        # vones: [kl(128), klt, sub-head(8), 128]: cols 0:64 = 1, 64:128 = V
        vones_sb = persist.tile([128, N_KLT, 8, 128], bf)
        wck_sb = persist.tile([128, 4, NKC, 128], bf)   # ht-major
        wcv_sb = persist.tile([128, NKC, HGD], bf)
        wq_sb = persist.tile([128, 8, HGD], f8)
        wo_sb = persist.tile([128, 4, D], bf)
        bk_sb = persist.tile([128, 4, 1], f32)
        bq_sb = persist.tile([128, 4, 1], f32)
        bvb_sb = persist.tile([128, HGD], f32)

        qt_tiles = {}

        def emit_q_dma(qc):
            q_sb = qstream.tile([128, 8, 512], f8, tag="q", name="q_sb")
            nc.sync.dma_start(
                out=q_sb,
                in_=qTn[qc * 128:(qc + 1) * 128, :].rearrange(
                    "p (i s) -> p i s", i=8))
            qt = qtp.tile([128, 4, 512], bf, tag="qt", name="qt")
            qt_tiles[qc] = (qt, q_sb)

        def emit_q_ht(qc, ht):
            qt, q_sb = qt_tiles[qc]
            psq = psP.tile([128, 512], f32, tag="proj", name="psq")
            if q_dr:
                for j in range(4):
                    nc.tensor.matmul(
                        psq,
                        wq_sb[:, 2 * j:2 * j + 2, ht * 128:(ht + 1) * 128],
                        q_sb[:, 2 * j:2 * j + 2, :],
                        start=(j == 0), stop=(j == 3), perf_mode=DR)
            else:
                for dm in range(8):
                    nc.tensor.matmul(
                        psq, wq_sb[:, dm, ht * 128:(ht + 1) * 128],
                        q_sb[:, dm, :], start=(dm == 0), stop=(dm == 7))
            nc.vector.tensor_scalar_add(
                qt[:, ht, :], psq, bq_sb[:, ht, :])

        def emit_kr_dma(ch):
            krb = krp.tile([128, NKC, KCH], bf, tag="kr", name="krb")
            nc.sync.dma_start(
                out=krb,
                in_=krTn[ch * 128:(ch + 1) * 128, :].rearrange(
                    "p (ci l) -> p ci l", ci=NKC))
            return krb

        def emit_wck_dma(ht):
            nc.sync.dma_start(
                out=wck_sb[:, ht],
                in_=wckTn[ht * 128:(ht + 1) * 128, :].rearrange(
                    "p (ci o) -> p ci o", ci=NKC))

        # ---- startup DMA order: all q chunks first (small, land fast,
        # and Q0-Q3 projections are the PE's only kv-independent work),
        # then kr/wck stream behind them.
        nc.sync.dma_start(
            out=wq_sb, in_=wqTn[:, :].rearrange("p (i s) -> p i s", i=8))
        emit_q_dma(0)
        emit_q_dma(1)
        nc.sync.dma_start(
            out=bq_sb, in_=bqf.rearrange("(t p) o -> p t o", p=128))
        emit_q_dma(2)
        emit_q_dma(3)
        kr_pf = {0: emit_kr_dma(0)}
        emit_wck_dma(0)
        kr_pf[1] = emit_kr_dma(1)
        for ht in range(1, 4):
            emit_wck_dma(ht)
        nc.sync.dma_start(
            out=bk_sb, in_=bkf.rearrange("(t p) o -> p t o", p=128))
        nc.sync.dma_start(out=bvb_sb, in_=bvf[0:1, :].partition_broadcast(128))
        kr_pf[2] = emit_kr_dma(2)

        for ht in range(4):
            emit_q_ht(0, ht)
        for ht in range(4):
            emit_q_ht(1, ht)
        for ht in range(4):
            emit_q_ht(2, ht)
        for ht in range(4):
            emit_q_ht(3, ht)

        nc.vector.memset(vones_sb[:, :, :, 0:64], 1.0)
        nc.vector.memset(ktT_sb[:, :, KL:KLP], 0.0)  # kl padding

        def emit_wcv_dma():
            # quarters so V(klt0)'s ci-chain can start early
            for c0 in range(0, NKC, 6):
                nc.sync.dma_start(
                    out=wcv_sb[:, c0:c0 + 6, :],
                    in_=wcvTn[:, c0 * HGD:(c0 + 6) * HGD].rearrange(
                        "p (ci o) -> p ci o", ci=6))

        def emit_wo_dma():
            nc.sync.dma_start(
                out=wo_sb, in_=woTn[:, :].rearrange("p (c d) -> p c d", c=4))

        # ---- K production: one KCH-col chunk, selectable head tiles ----
        kr_live = {}

        def emit_k(ch, hts=(0, 1, 2, 3)):
            kc0 = ch * KCH
            kcn = min(KCH, KL - kc0)
            if ch in kr_live:
                krb = kr_live[ch]
            elif ch in kr_pf:
                krb = kr_live[ch] = kr_pf.pop(ch)
            else:
                krb = kr_live[ch] = emit_kr_dma(ch)
            for ht in hts:
                psk = psP.tile([128, 512], f32, tag="proj", name="psk")
                for ci in range(NKC):
                    nc.tensor.matmul(
                        psk[:, :kcn],
                        wck_sb[:, ht, ci, :],
                        krb[:, ci, :kcn],
                        start=(ci == 0), stop=(ci == NKC - 1))
                nc.vector.tensor_scalar_add(
                    ktT_sb[:, ht, kc0:kc0 + kcn],
                    psk[:, :kcn], bk_sb[:, ht, :])

        def emit_v(klt):
            rn = min(128, KL - klt * 128)
            vrb = vrp.tile([128, NKC, 128], bf, tag="vr", name="vrb")
            nc.sync.dma_start(
                out=vrb,
                in_=vrTn[klt * 128:(klt + 1) * 128, :].rearrange(
                    "p (ci l) -> p ci l", ci=NKC))
            psv = psP.tile([128, 512], f32, tag="proj", name="psv")
            for ci in range(NKC):
                nc.tensor.matmul(
                    psv[:rn, :], vrb[:, ci, :rn], wcv_sb[:, ci, :],
                    start=(ci == 0), stop=(ci == NKC - 1))
            nc.vector.tensor_tensor(
                out=vones_sb[:rn, klt, :, 64:128],
                in0=psv[:rn].rearrange("p (h c) -> p h c", h=8),
                in1=bvb_sb[:rn].rearrange("p (h c) -> p h c", h=8),
                op=mybir.AluOpType.add)

        # K/V emission cursors for just-in-time production
        kv_state = {"k": 0, "v": 0, "k0_rest": False}

        def need_k(ch):
            if kv_state["k"] == 0:
                emit_k(0, hts=(0,))
                kv_state["k"] = 1
            if ch >= 1 and not kv_state["k0_rest"]:
                emit_k(0, hts=(1, 2, 3))
                del kr_live[0]
                kv_state["k0_rest"] = True
            while kv_state["k"] <= min(ch, NKCH - 1):
                emit_k(kv_state["k"])
                del kr_live[kv_state["k"]]
                kv_state["k"] += 1

        def need_v(klt):
            if kv_state["v"] == 0:
                emit_wcv_dma()
            while kv_state["v"] <= min(klt, N_KLT - 1):
                emit_v(kv_state["v"])
                kv_state["v"] += 1

        # ---- attention unit: head pair hp x q-chunk qc ----
        def emit_unit(hp, qc, qt, osb_t, kv_jit=False, pop=None):
            ht = hp
            psos = [psO.tile([128, 512], f32, tag=f"pso{s}", name=f"pso{s}")
                    for s in range(2)]
            pts = {}

            def emit_av(k):
                rn = min(128, KL - k * 128)
                pt = pts.pop(k)
                for sub in range(2):
                    nc.tensor.matmul(
                        psos[sub],
                        vones_sb[:rn, k, 2 * hp + sub, :],
                        pt[:rn, sub * 512:(sub + 1) * 512],
                        start=(k == 0), stop=(k == N_KLT - 1))

            for klt in range(N_KLT):
                if kv_jit:
                    need_k(klt * 128 // KCH)
                if pop_pre and pop is not None and klt % pop_mod == 1:
                    pop()
                pss = psA.tile([128, 1024], f32, tag="pss", name="pss")
                for sub in range(2):
                    hb = sub * 64
                    nc.tensor.matmul(
                        pss[:, sub * 512:(sub + 1) * 512],
                        ktT_sb[hb:hb + 64, ht,
                               klt * 128:(klt + 1) * 128],
                        qt[hb:hb + 64, ht, :],
                        start=True, stop=True)
                pt = ptp.tile([128, 1024], bf, tag="pt", name="pt")
                nc.scalar.activation(pt, pss, EXP, scale=0.125)
                pts[klt] = pt
                if klt >= lag:
                    if kv_jit:
                        need_v(klt - lag)
                    emit_av(klt - lag)
                if not pop_pre and pop is not None and klt % pop_mod == 1:
                    pop()
            if kv_jit:
                need_v(N_KLT - 1)
            for k in range(max(0, N_KLT - lag), N_KLT):
                emit_av(k)

            for sub in range(2):
                hb = sub * 64
                # den is pre-broadcast on psum partitions 0:64 (ones cols
                # lead); x sits on partitions 64:128. recip reads PSUM
                # base-0 directly, so the staging copy is gone.
                rcb = stgp.tile([64, 512], f32, tag=f"rcb{sub}",
                                name=f"rcb{sub}")
                nc.vector.reciprocal_approx_fast(out=rcb,
                                                 in_=psos[sub][0:64, :])
                nc.vector.tensor_tensor(
                    out=osb_t[hb:hb + 64, ht, :],
                    in0=psos[sub][64:128, :], in1=rcb,
                    op=mybir.AluOpType.mult)

        # ---- fine-grained filler closures ----
        def o_filler_closures(qc, osb_t, alt_pools=False):
            qsl = slice(qc * 512, (qc + 1) * 512)
            state = {}

            def mk_mm(dt, hc):
                def f():
                    if hc == 0:
                        if alt_pools and dt % 2 == 1:
                            state[dt] = psO.tile(
                                [128, 512], f32,
                                tag=f"pso{(dt // 2) % 2}", name="pp8")
                        else:
                            state[dt] = psP.tile([128, 512], f32,
                                                 tag="proj", name="pp")
                    nc.tensor.matmul(
                        state[dt], wo_sb[:, hc, dt * 128:(dt + 1) * 128],
                        osb_t[:, hc, :], start=(hc == 0), stop=(hc == 3))
                return f

            def mk_drain(dt):
                def f():
                    ot = otp.tile([128, 512], bf, tag="ot", name="ot")
                    nc.vector.tensor_copy(ot, state.pop(dt))
                    nc.sync.dma_start(
                        out=outT[dt * 128:(dt + 1) * 128, qsl], in_=ot)
                return f

            out = []
            for dt in range(8):
                for hc in range(4):
                    out.append(mk_mm(dt, hc))
                out.append(mk_drain(dt))
            return out

        def q_filler_closures(qc):
            state = {}
            NJ = 4 if q_dr else 8

            def mk_mm(ht, j):
                def f():
                    if j == 0:
                        state[ht] = psP.tile([128, 512], f32, tag="proj",
                                             name="psq")
                    if q_dr:
                        nc.tensor.matmul(
                            state[ht],
                            wq_sb[:, 2 * j:2 * j + 2,
                                  ht * 128:(ht + 1) * 128],
                            qt_tiles[qc][1][:, 2 * j:2 * j + 2, :],
                            start=(j == 0), stop=(j == NJ - 1),
                            perf_mode=DR)
                    else:
                        nc.tensor.matmul(
                            state[ht], wq_sb[:, j, ht * 128:(ht + 1) * 128],
                            qt_tiles[qc][1][:, j, :],
                            start=(j == 0), stop=(j == NJ - 1))
                return f

            def mk_drain(ht):
                def f():
                    nc.vector.tensor_scalar_add(
                        qt_tiles[qc][0][:, ht, :], state.pop(ht),
                        bq_sb[:, ht, :])
                    if ht == 3:
                        q_done[qc] = True
                return f

            out = [] if qc in qt_tiles else [lambda: emit_q_dma(qc)]
            for ht in range(4):
                for j in range(NJ):
                    out.append(mk_mm(ht, j))
                out.append(mk_drain(ht))
            return out

        # ---- main stream: units with O/Q filler weaving ----
        from collections import deque
        fillers = deque()
        q_done = {0: True, 1: True, 2: True, 3: True}

        def pop():
            for _ in range(pop_n):
                if fillers:
                    fillers.popleft()()

        need_k(0)
        fillers.append(emit_wo_dma)
        fillers.extend(q_filler_closures(4))
        fillers.extend(q_filler_closures(5))
        fillers.extend(q_filler_closures(6))

        for qc in range(NQC):
            while not q_done.get(qc):
                pop()
            osb_t = osbp.tile([128, 4, 512], bf, tag="osb", name="osb")
            for hp in range(4):
                kv_jit = (qc == 0 and hp == 0)
                emit_unit(hp, qc, qt_tiles[qc][0], osb_t,
                          kv_jit=kv_jit, pop=pop)
            for f in o_filler_closures(qc, osb_t,
                                       alt_pools=(qc == NQC - 1)):
                fillers.append(f)
            nq = qc + q_ahead + 1
            if nq < NQC:
                fillers.extend(q_filler_closures(nq))
        while fillers:
            pop()

    nc.finalize()
    return nc


def _host_inputs(inputs):
    """Build the 8 per-core input maps from full fp32 inputs."""
    q32 = np.asarray(inputs["query"], np.float32)
    k32 = np.asarray(inputs["key"], np.float32)
    v32 = np.asarray(inputs["value"], np.float32)
    Wq, bq = np.asarray(inputs["Wq"], np.float32), np.asarray(inputs["bq"], np.float32)
    Wk, bk = np.asarray(inputs["Wk"], np.float32), np.asarray(inputs["bk"], np.float32)
    Wv, bv = np.asarray(inputs["Wv"], np.float32), np.asarray(inputs["bv"], np.float32)
    Wo = np.asarray(inputs["Wo"], np.float32)
    conv_w = np.asarray(inputs["conv_w"], np.float32)
    conv_b = np.asarray(inputs["conv_b"], np.float32)

    Wc = conv_w.transpose(2, 1, 0).reshape(CD, D)  # [3072, 1024]

    per_hg = []
    for hg in range(2):
        hsl = slice(hg * HGD, (hg + 1) * HGD)
        wck = (Wc @ Wk[hsl].T).astype(bf16)    # [CD, HGD]
        wcv = (Wc @ Wv[hsl].T).astype(bf16)
        wq = Wq[hsl].T.astype(e4m3)            # [D, HGD]
        wo = Wo[:, hsl].T.astype(bf16)         # [HGD, D]
        # wckTn [4*128, NKC*128]: row ht*128+p, col ci*128+o
        wckTn = np.ascontiguousarray(
            wck.reshape(NKC, 128, 4, 128).transpose(2, 1, 0, 3)
            .reshape(4 * 128, NKC * 128))
        # wcvTn [128, NKC*HGD]: row p, col ci*HGD+o
        wcvTn = np.ascontiguousarray(
            wcv.reshape(NKC, 128, HGD).transpose(1, 0, 2)
            .reshape(128, NKC * HGD))
        # wqTn [128, 8*HGD]: row p, col dm*HGD+o
        wqTn = np.ascontiguousarray(
            wq.reshape(8, 128, HGD).transpose(1, 0, 2).reshape(128, 8 * HGD))
        # woTn [128, 4*D]: row p, col c*D+d
        woTn = np.ascontiguousarray(
            wo.reshape(4, 128, D).transpose(1, 0, 2).reshape(128, 4 * D))
        per_hg.append(dict(
            wckTn=wckTn, wcvTn=wcvTn, wqTn=wqTn, woTn=woTn,
            bqf=bq[hsl].reshape(HGD, 1).astype(np.float32),
            bkf=(conv_b @ Wk[hsl].T + bk[hsl]).reshape(HGD, 1).astype(np.float32),
            bvf=(conv_b @ Wv[hsl].T + bv[hsl]).reshape(1, HGD).astype(np.float32),
        ))

    per_b = []
    zpad = np.zeros((PAD, D), np.float32)
    for b in range(B):
        xr_k = np.concatenate([zpad, k32[b]], 0).reshape(KL, CD)
        xr_v = np.concatenate([zpad, v32[b]], 0).reshape(KL, CD)
        # krTn [NKCH*128, NKC*KCH]: row ch*128+p, col ci*KCH+l
        # (kr.T is [CD, KL] with row ci*128+p)
        krT = xr_k.T.astype(bf16)              # [CD, KL]
        krpk = np.zeros((NKCH * 128, NKC * KCH), bf16)
        for ch in range(NKCH):
            kcn = min(KCH, KL - ch * KCH)
            blk = krT[:, ch * KCH:ch * KCH + kcn].reshape(NKC, 128, kcn)
            krpk[ch * 128:(ch + 1) * 128]\
                .reshape(128, NKC, KCH)[:, :, :kcn] = blk.transpose(1, 0, 2)
        vrT = xr_v.T.astype(bf16)
        vrpk = np.zeros((N_KLT * 128, NKC * 128), bf16)
        for klt in range(N_KLT):
            rn = min(128, KL - klt * 128)
            blk = vrT[:, klt * 128:klt * 128 + rn].reshape(NKC, 128, rn)
            vrpk[klt * 128:(klt + 1) * 128]\
                .reshape(128, NKC, 128)[:, :, :rn] = blk.transpose(1, 0, 2)
        # qTn [NQC*128, 8*512]: row qc*128+p, col dm*512+s
        qT = q32[b].T.astype(e4m3)             # [D, S]
        qTn = np.ascontiguousarray(
            qT.reshape(8, 128, NQC, 512).transpose(2, 1, 0, 3)
            .reshape(NQC * 128, 8 * 512))
        per_b.append(dict(qTn=qTn, krTn=krpk, vrTn=vrpk))

    in_maps = []
    for c in range(8):
        b, hg = c // 2, c % 2
        in_maps.append({**per_b[b], **per_hg[hg]})
    return in_maps


def kernel(**inputs):
    from concourse.bass_utils import run_bass_kernel_spmd

    if "nc" not in _CACHE:
        _CACHE["nc"] = _build_nc()
    nc = _CACHE["nc"]

    in_maps = _host_inputs(inputs)
    r = run_bass_kernel_spmd(nc, in_maps, list(range(8)))
    _CACHE["exec_time_ns"] = r.exec_time_ns
    _CACHE["result"] = r
    res = r.results

    bo = np.asarray(inputs["bo"], np.float32)
    out = np.empty((B, S, D), np.float32)
    for b in range(B):
        out[b] = (res[2 * b]["outT"].astype(np.float32).T
                  + res[2 * b + 1]["outT"].astype(np.float32).T + bo)
    return out
